# revision 18
# baseline (speedup 1.0000x reference)
"""BNN MNIST MLP on 8 Trainium2 NeuronCores — pure data parallel.

Model (inference): x[B,784] -> relu(x @ sign(W1)) -> BN1 -> sign ->
@ sign(W2) relu BN2 sign -> @ sign(W3) -> softmax.

Key transformations:
  * BN(relu(h)) >= 0  <=>  h >= t  (per-feature threshold t, since BN scale>0),
    so each binarize step is one ScalarE Sign(h - t) op straight from PSUM.
  * Layer-1 precision: x is split on host into an fp16 hi part plus an
    fp8-e3m4 residual lo8 = e3m4((x - fp16(x)) * 2^12); the 2^-12 decode
    scale is folded into the (fp16) stationary sign-weights of the lo
    matmul chunks, so hi and lo products accumulate into one fp32 PSUM
    group with no extra vector work. 3 bytes/element instead of 4 —
    measured end-to-end rel_l2 vs the fp32 reference is ~4.6e-3 (one
    sign-flip row in 65536), far inside the 2e-2 gate, while HBM traffic
    (the roofline term) drops 25%.
  * DMA shape tuned by micro-bench: always 128 SBUF partitions (112-wide
    transfers leave 4 of the 16 SDMA engines half-idle) and ~4 KB
    contiguous per partition per dma_start (the per-engine sweet spot —
    one big descriptor per partition is HBM-latency-bound at ~13 GB/s,
    4 KB slices pipeline at ~21+). Hence 6 contraction chunks of 128
    features, slab-blocked contiguous in DRAM, loaded as column-sliced
    dma_starts alternating between the two HWDGE rings; the 16 leftover
    features ride in small whole-core [16, 8192] tensors loaded upfront.
  * The hidden width (50) uses only half the PE array columns, so the two
    half-slab groups run CONCURRENTLY via column tiling (tile_position
    (0,0) / (0,64)) — halving layer-1 streaming time.
  * The slab loop is software-pipelined (A(p) | B(p-1) | CD(p-2)) and the
    last two slabs are 512 rows instead of 1024, so the post-stream drain
    (which runs on a HAM-cold 1.2 GHz PE) is a few us instead of ~20.
  * Layer 3 is fused with the output transpose: its stationary operand is
    a stride-8 batch pick of s2, so each matmul emits batch-major logits
    directly into PSUM — no PSUM->SBUF logit copy and no separate PE
    transpose pass; softmax runs straight on the PSUM tile. Output rows
    are padded to 16 floats (512 B per partition per store, no sub-512B
    RMW descriptors); the host strips the pad after the gather.
"""
import numpy as np
import ml_dtypes

import concourse.mybir as mybir
from concourse import bacc
from concourse.tile import TileContext
from concourse.bass_utils import run_bass_kernel_spmd

F32 = mybir.dt.float32
F16 = mybir.dt.float16
F8E3 = mybir.dt.float8e3

B = 65536
NCORES = 8
PER = B // NCORES          # 8192 rows per core
SLABS = [1024] * 7 + [512, 512]
NSLAB = len(SLABS)
B0 = [sum(SLABS[:i]) for i in range(NSLAB)]
K = 784
NCH = 6                    # full 128-partition contraction chunks
KT = K - NCH * 128         # 16 tail features, whole-core upfront load
NCLS = 10
NF = 16                    # padded output row (f32) -> 512B store descs
NHID = 50
LOSHIFT = 12               # lo8 = e3m4(lo * 2^12); weights carry 2^-12
PF = 3                     # emission lead. HWDGE flow control admits only
                           # ~16 outstanding DMAs (2/sem-lane x 8 lanes), so
                           # a dma_start issued further ahead BLOCKS its
                           # engine queue - and the ops behind it in that
                           # engine FIFO convoy the whole pipeline. 3 slabs
                           # (~15 DMAs) is exactly the admissible runway.
HICOL = 2048               # dma_start column slice: hi 2048*2B = 4KB/part
LOCOL = 4096               # lo 4096*1B = 4KB/part
NWARM = 12                 # PE warm-up matmuls (~5.7us at the cold clock,
                           # covers one aligned HAM window)

EPS = 1e-3

_CACHE = {}


def _build():
    nc = bacc.Bacc("TRN2", target_bir_lowering=False, debug=False,
                   num_devices=NCORES)

    xhi = nc.dram_tensor("xhi", [128, NCH * PER], F16,
                         kind="ExternalInput").ap()
    xlo = nc.dram_tensor("xlo", [128, NCH * PER], F8E3,
                         kind="ExternalInput").ap()
    xhit = nc.dram_tensor("xhit", [KT, PER], F16, kind="ExternalInput").ap()
    xlot = nc.dram_tensor("xlot", [KT, PER], F8E3, kind="ExternalInput").ap()
    # fp16 consts: w1-hi chunks at cols [50c, 50c+50) (tail chunk at c=6,
    # rows 0:16), w1-lo (pre-scaled by 2^-12) at [350+50c, ...), w2 at
    # [700, 750), w3 at [750, 760); w2/w3 replicated at partition 64 for
    # the column-tiled pair.
    cb16 = nc.dram_tensor("cb16", [128, 760], F16, kind="ExternalInput").ap()
    # fp32 consts: col 0 = -T1, col 1 = -T2 (replicated at partition 64)
    cb32 = nc.dram_tensor("cb32", [128, 2], F32, kind="ExternalInput").ap()
    out = nc.dram_tensor("out", [PER, NF], F32, kind="ExternalOutput").ap()

    W2C = 700
    W3C = 750

    with TileContext(nc) as tc:
        with (
            tc.tile_pool(name="consts", bufs=1) as cpool,
            tc.tile_pool(name="xin", bufs=PF + 3) as xpool,
            tc.tile_pool(name="mid", bufs=3) as mpool,
            tc.tile_pool(name="fin", bufs=6) as fpool,
            tc.tile_pool(name="psA", bufs=2, space="PSUM") as psA,
            tc.tile_pool(name="psB", bufs=3, space="PSUM") as psB,
            tc.tile_pool(name="psW", bufs=1, space="PSUM") as psW,
        ):
            cb16t = cpool.tile([128, 760], F16, tag="cb16")
            nc.sync.dma_start(cb16t[:], cb16[:, :])
            cb32t = cpool.tile([128, 2], F32, tag="cb32")
            nc.scalar.dma_start(cb32t[:], cb32[:, :])
            # whole-core tail-feature tensors (loaded after the first two
            # slabs - their chunks run last in each accumulation)
            hit = cpool.tile([KT, PER], F16, tag="hit")
            lot = cpool.tile([KT, PER], F8E3, tag="lot")

            def emit_tail_consts():
                # one dma_start each: only 16 partitions and ~384 KB total,
                # and using 2 instead of 6 issues keeps the upfront burst
                # inside the ~16-DMA HWDGE flow-control window so the
                # follow-on slab issues release back-to-back
                nc.sync.dma_start(hit[:], xhit[:, :])
                nc.scalar.dma_start(lot[:], xlot[:, :])

            w1hi = [cb16t[0:128, c * NHID:(c + 1) * NHID] for c in range(NCH)]
            w1hit = cb16t[0:KT, NCH * NHID:(NCH + 1) * NHID]
            w1lo = [cb16t[0:128, 350 + c * NHID:350 + (c + 1) * NHID]
                    for c in range(NCH)]
            w1lot = cb16t[0:KT, 350 + NCH * NHID:350 + (NCH + 1) * NHID]
            w2t = cb16t[0:NHID, W2C:W2C + NHID]
            w3t = cb16t[0:NHID, W3C:W3C + NCLS]
            w2t64 = cb16t[64:64 + NHID, W2C:W2C + NHID]
            w3t64 = cb16t[64:64 + NHID, W3C:W3C + NCLS]
            nt1t = cb32t[0:64 + NHID, 0:1]
            nt2t = cb32t[0:64 + NHID, 1:2]

            # HAM keeps the PE array clock-gated to 1.2 GHz until it has
            # seen ~3.4us of sustained matmul activity; burn that in on
            # junk data during the DMA fill window so the real pipeline
            # runs at 2.4 GHz from slab 0.
            psw = psW.tile([64, 512], F32, tag="psw")
            for _ in range(NWARM):
                nc.tensor.matmul(psw[0:NHID, :], w2t, cb16t[0:NHID, 0:512],
                                 start=True, stop=True, skip_group_check=True)

            xt = {}
            s1t = {}
            s2v = {}
            ring = [0]

            def emit_loads(s):
                n = SLABS[s]
                off = NCH * B0[s]
                tot = NCH * n
                engs = (nc.sync, nc.scalar)
                th = xpool.tile([128, NCH * 1024], F16, tag="xh",
                                name=f"xh{s}")
                tl = xpool.tile([128, NCH * 1024], F8E3, tag="xl",
                                name=f"xl{s}")
                for c0 in range(0, tot, HICOL):
                    c1 = min(c0 + HICOL, tot)
                    engs[ring[0] % 2].dma_start(th[:, c0:c1],
                                                xhi[:, off + c0:off + c1])
                    ring[0] += 1
                for c0 in range(0, tot, LOCOL):
                    c1 = min(c0 + LOCOL, tot)
                    engs[ring[0] % 2].dma_start(tl[:, c0:c1],
                                                xlo[:, off + c0:off + c1])
                    ring[0] += 1
                xt[s] = (th, tl)

            def stageA(p):
                # one slab = 2 groups of n/2 rows run CONCURRENTLY on the PE
                # via column tiling: group 0 on array columns 0-63 (out
                # partitions 0-49), group 1 on columns 64-127 (out
                # partitions 64-113). 6+1 fp16-hi chunks then 6+1 fp8-lo
                # chunks accumulate into one PSUM group.
                n = SLABS[p]
                g = n // 2
                b0 = B0[p]
                th, tl = xt[p]
                ps1 = psA.tile([128, 512], F32, tag="ps1")
                hi_ops = [(w1hi[c], th[0:128, c * n:c * n + g],
                           th[0:128, c * n + g:(c + 1) * n])
                          for c in range(NCH)]
                lo_ops = [(w1lo[c], tl[0:128, c * n:c * n + g],
                           tl[0:128, c * n + g:(c + 1) * n])
                          for c in range(NCH)]
                ops = hi_ops + lo_ops + [
                    (w1hit, hit[:, b0:b0 + g], hit[:, b0 + g:b0 + n]),
                    (w1lot, lot[:, b0:b0 + g], lot[:, b0 + g:b0 + n])]
                for i, (w, ma, mb) in enumerate(ops):
                    nc.tensor.matmul(ps1[0:NHID, 0:g], w, ma,
                                     start=(i == 0), stop=(i == len(ops) - 1),
                                     skip_group_check=True)
                    nc.tensor.matmul(ps1[64:64 + NHID, 0:g], w, mb,
                                     start=(i == 0), stop=(i == len(ops) - 1),
                                     skip_group_check=True)
                s1 = mpool.tile([64 + NHID, 512], F16, tag="s1", name=f"s1_{p}")
                nc.scalar.sign(s1[:, 0:g], ps1[0:64 + NHID, 0:g], bias=nt1t)
                s1t[p] = (s1[0:NHID, 0:g], s1[64:64 + NHID, 0:g])

            def stageB(p):
                g = SLABS[p] // 2
                ps2 = psA.tile([128, 512], F32, tag="ps2")
                sa, sb = s1t[p]
                nc.tensor.matmul(ps2[0:NHID, 0:g], w2t, sa,
                                 start=True, stop=True, skip_group_check=True)
                nc.tensor.matmul(ps2[64:64 + NHID, 0:g], w2t64, sb,
                                 start=True, stop=True, skip_group_check=True)
                s2 = mpool.tile([64 + NHID, 512], F16, tag="s2", name=f"s2_{p}")
                nc.scalar.sign(s2[:, 0:g], ps2[0:64 + NHID, 0:g], bias=nt2t)
                v = s2[:, 0:g].rearrange("q (j r) -> q j r", r=8)
                s2v[p] = (v[0:NHID, :, :], v[64:64 + NHID, :, :])

            def stageCD(p):
                # Layer 3 fused with the output transpose: the stationary
                # operand is a stride-8 batch pick of s2, so out partition q
                # holds batch rows {8q + r} of the slab -> contiguous
                # batch-major store, no PSUM->SBUF logit copy and no PE
                # transpose pass.
                n = SLABS[p]
                g = n // 2
                j = g // 8
                ps4 = psB.tile([128, 8 * NCLS], F32, tag="ps4", name=f"ps4_{p}")
                s2a3, s2b3 = s2v[p]
                for r in range(8):
                    nc.tensor.matmul(ps4[0:j, r * NCLS:(r + 1) * NCLS],
                                     s2a3[:, :, r], w3t,
                                     start=True, stop=True,
                                     skip_group_check=True)
                    nc.tensor.matmul(ps4[64:64 + j, r * NCLS:(r + 1) * NCLS],
                                     s2b3[:, :, r], w3t64,
                                     start=True, stop=True,
                                     skip_group_check=True)
                eo = fpool.tile([128, 8 * NCLS], F32, tag="eo", name=f"eo_{p}")
                nc.scalar.activation(eo[:], ps4[:],
                                     mybir.ActivationFunctionType.Exp)
                sm = fpool.tile([128, 8], F32, tag="sm", name=f"sm_{p}")
                eov = eo[:].rearrange("q (r c) -> q r c", c=NCLS)
                nc.vector.tensor_reduce(sm[:], eov, axis=mybir.AxisListType.X,
                                        op=mybir.AluOpType.add)
                rv = fpool.tile([128, 8], F32, tag="rv", name=f"rv_{p}")
                nc.vector.reciprocal(rv[:], sm[:])
                ot = fpool.tile([128, 8 * NF], F32, tag="ot", name=f"ot_{p}")
                otv = ot[:].rearrange("q (r c) -> q r c", c=NF)
                rvb = rv[:].unsqueeze(-1).broadcast_to([128, 8, NCLS])
                nc.vector.tensor_mul(otv[:, :, 0:NCLS], eov, rvb)
                b0 = B0[p]
                eng = nc.sync if p % 2 == 0 else nc.scalar
                if n == 1024:
                    dst = out[b0:b0 + n, :].rearrange("(q r) f -> q (r f)",
                                                      q=128)
                    eng.dma_start(dst, ot[:])
                else:
                    # group 1 lives at partitions 0..j-1, group 2 at 64..63+j
                    d1 = out[b0:b0 + g, :].rearrange("(q r) f -> q (r f)", q=j)
                    d2 = out[b0 + g:b0 + n, :].rearrange("(q r) f -> q (r f)",
                                                         q=j)
                    eng.dma_start(d1, ot[0:j, :])
                    eng.dma_start(d2, ot[64:64 + j, :])

            # software pipeline: A(p) streams while ScalarE signs slab p-1
            # and the L3/softmax of slab p-2 fills PE/DVE slack; the two
            # 512-row tail slabs keep the post-stream drain short.
            emit_loads(0)
            emit_loads(1)
            emit_tail_consts()
            for s in range(2, PF):
                emit_loads(s)
            # loads are emitted LAST in each tick: the tick's ACTIVATEs
            # must precede them in the scalar FIFO, since a dma_start can
            # block on HWDGE flow control and convoy everything behind it
            for p in range(NSLAB - 2):
                stageA(p)
                if p >= 1:
                    stageB(p - 1)
                if p >= 2:
                    stageCD(p - 2)
                if p + PF < NSLAB:
                    emit_loads(p + PF)
            stageA(NSLAB - 2)
            stageB(NSLAB - 3)
            stageCD(NSLAB - 4)
            stageA(NSLAB - 1)
            stageB(NSLAB - 2)
            stageCD(NSLAB - 3)
            stageB(NSLAB - 1)
            stageCD(NSLAB - 2)
            stageCD(NSLAB - 1)

    nc.compile()
    return nc


def _swizzle(xt, dtype):
    """[768, 8192] feature-major block (main chunks) -> [128, 6*8192]
    slab-blocked layout: per slab s a contiguous [128, 6*len] block whose
    column j = c*len + i holds feature c*128+p, batch B0[s]+i."""
    r = xt.reshape(NCH, 128, PER)
    blocks = [np.ascontiguousarray(
        r[:, :, B0[s]:B0[s] + SLABS[s]].transpose(1, 0, 2)
    ).reshape(128, NCH * SLABS[s]) for s in range(NSLAB)]
    return np.ascontiguousarray(np.concatenate(blocks, axis=1), dtype=dtype)


def _prep_host(inputs, W1, W2, W3, g1, b1, m1, v1, g2, b2, m2, v2):
    x = np.ascontiguousarray(inputs.reshape(B, K).astype(np.float32, copy=False))
    xhi = x.astype(np.float16)
    xlo8 = ((x - xhi.astype(np.float32)) * float(2 ** LOSHIFT)) \
        .astype(ml_dtypes.float8_e3m4)

    w1b = np.where(W1 >= 0, 1.0, -1.0).astype(np.float16)
    w2b = np.where(W2 >= 0, 1.0, -1.0).astype(np.float16)
    w3b = np.where(W3 >= 0, 1.0, -1.0).astype(np.float16)

    a1 = g1.astype(np.float64) / np.sqrt(v1.astype(np.float64) + EPS)
    c1 = b1.astype(np.float64) - a1 * m1.astype(np.float64)
    t1 = -c1 / a1
    T1 = np.where(t1 > 0, t1, -1e30).astype(np.float32)
    a2 = g2.astype(np.float64) / np.sqrt(v2.astype(np.float64) + EPS)
    c2 = b2.astype(np.float64) - a2 * m2.astype(np.float64)
    t2 = -c2 / a2
    T2 = np.where(t2 > 0, t2, -1e30).astype(np.float32)

    lscale = np.float16(2.0 ** -LOSHIFT)
    cb16 = np.zeros((128, 760), dtype=np.float16)
    for c in range(NCH):
        cb16[:, c * NHID:(c + 1) * NHID] = w1b[c * 128:(c + 1) * 128]
        cb16[:, 350 + c * NHID:350 + (c + 1) * NHID] = \
            w1b[c * 128:(c + 1) * 128] * lscale
    cb16[:KT, NCH * NHID:(NCH + 1) * NHID] = w1b[NCH * 128:]
    cb16[:KT, 350 + NCH * NHID:350 + (NCH + 1) * NHID] = w1b[NCH * 128:] * lscale
    cb16[:NHID, 700:750] = w2b
    cb16[:NHID, 750:760] = w3b
    cb16[64:64 + NHID, 700:750] = w2b
    cb16[64:64 + NHID, 750:760] = w3b
    cb32 = np.zeros((128, 2), dtype=np.float32)
    cb32[:NHID, 0] = -T1
    cb32[64:64 + NHID, 0] = -T1
    cb32[:NHID, 1] = -T2
    cb32[64:64 + NHID, 1] = -T2
    shared = {"cb16": cb16, "cb32": cb32}
    in_maps = []
    for c in range(NCORES):
        sl = slice(c * PER, (c + 1) * PER)
        m = dict(shared)
        hiT = np.ascontiguousarray(xhi[sl].T)       # [784, 8192]
        loT = np.ascontiguousarray(xlo8[sl].T)
        m["xhi"] = _swizzle(hiT[:NCH * 128], np.float16)
        m["xlo"] = _swizzle(loT[:NCH * 128], ml_dtypes.float8_e3m4)
        m["xhit"] = np.ascontiguousarray(hiT[NCH * 128:])
        m["xlot"] = np.ascontiguousarray(loT[NCH * 128:])
        in_maps.append(m)
    return in_maps


def kernel(**inputs):
    if "nc" not in _CACHE:
        _CACHE["nc"] = _build()
    nc = _CACHE["nc"]
    inputs = {k: np.asarray(v) for k, v in inputs.items()}
    in_maps = _prep_host(**inputs)
    res = run_bass_kernel_spmd(nc, in_maps, core_ids=list(range(NCORES)))
    return np.ascontiguousarray(
        np.concatenate([r["out"] for r in res.results], axis=0)[:, :NCLS])


# revision 20
# speedup vs baseline: 1.0741x; 1.0741x over previous
"""BNN MNIST MLP on 8 Trainium2 NeuronCores — pure data parallel.

Model (inference): x[B,784] -> relu(x @ sign(W1)) -> BN1 -> sign ->
@ sign(W2) relu BN2 sign -> @ sign(W3) -> softmax.

Key transformations:
  * BN(relu(h)) >= 0  <=>  h >= t  (per-feature threshold t, since BN scale>0),
    so each binarize step is one ScalarE Sign(h - t) op straight from PSUM.
  * Layer-1 precision: x is split on host into an fp16 hi part plus an
    fp8-e3m4 residual lo8 = e3m4((x - fp16(x)) * 2^12); the 2^-12 decode
    scale is folded into the (fp16) stationary sign-weights of the lo
    matmul chunks, so hi and lo products accumulate into one fp32 PSUM
    group with no extra vector work. 3 bytes/element instead of 4 —
    measured end-to-end rel_l2 vs the fp32 reference is ~4.6e-3 (one
    sign-flip row in 65536), far inside the 2e-2 gate, while HBM traffic
    (the roofline term) drops 25%.
  * DMA shape tuned by micro-bench: always 128 SBUF partitions (112-wide
    transfers leave 4 of the 16 SDMA engines half-idle) and ~4 KB
    contiguous per partition per dma_start (the per-engine sweet spot —
    one big descriptor per partition is HBM-latency-bound at ~13 GB/s,
    4 KB slices pipeline at ~21+). Hence 6 contraction chunks of 128
    features, slab-blocked contiguous in DRAM, loaded as column-sliced
    dma_starts alternating between the two HWDGE rings; the 16 leftover
    features ride in small whole-core [16, 8192] tensors loaded upfront.
  * The hidden width (50) uses only half the PE array columns, so the two
    half-slab groups run CONCURRENTLY via column tiling (tile_position
    (0,0) / (0,64)) — halving layer-1 streaming time.
  * The slab loop is software-pipelined (A(p) | B(p-1) | CD(p-2)) and the
    last two slabs are 512 rows instead of 1024, so the post-stream drain
    (which runs on a HAM-cold 1.2 GHz PE) is a few us instead of ~20.
  * Layer 3 is fused with the output transpose: its stationary operand is
    a stride-8 batch pick of s2, so each matmul emits batch-major logits
    directly into PSUM — no PSUM->SBUF logit copy and no separate PE
    transpose pass; softmax runs straight on the PSUM tile. Output rows
    are padded to 16 floats (512 B per partition per store, no sub-512B
    RMW descriptors); the host strips the pad after the gather.
"""
import numpy as np
import ml_dtypes

import concourse.mybir as mybir
from concourse import bacc
from concourse.tile import TileContext
from concourse.bass_utils import run_bass_kernel_spmd

F32 = mybir.dt.float32
F16 = mybir.dt.float16
F8E3 = mybir.dt.float8e3

B = 65536
NCORES = 8
PER = B // NCORES          # 8192 rows per core
SLABS = [1024] * 7 + [512, 512]
NSLAB = len(SLABS)
B0 = [sum(SLABS[:i]) for i in range(NSLAB)]
K = 784
NCH = 6                    # full 128-partition contraction chunks
KT = K - NCH * 128         # 16 tail features, whole-core upfront load
NCLS = 10
NF = 16                    # padded output row (f32) -> 512B store descs
NHID = 50
LOSHIFT = 12               # lo8 = e3m4(lo * 2^12); weights carry 2^-12
PF = 3                     # emission lead. HWDGE flow control admits only
                           # ~16 outstanding DMAs (2/sem-lane x 8 lanes), so
                           # a dma_start issued further ahead BLOCKS its
                           # engine queue - and the ops behind it in that
                           # engine FIFO convoy the whole pipeline. 3 slabs
                           # (~15 DMAs) is exactly the admissible runway.
HICOL = 2048               # dma_start column slice: hi 2048*2B = 4KB/part
LOCOL = 4096               # lo 4096*1B = 4KB/part
NWARM = 12                 # PE warm-up matmuls (~5.7us at the cold clock,
                           # covers one aligned HAM window)

EPS = 1e-3

_CACHE = {}


def _build():
    nc = bacc.Bacc("TRN2", target_bir_lowering=False, debug=False,
                   num_devices=NCORES)

    xhi = nc.dram_tensor("xhi", [128, NCH * PER], F16,
                         kind="ExternalInput").ap()
    xlo = nc.dram_tensor("xlo", [128, NCH * PER], F8E3,
                         kind="ExternalInput").ap()
    xhit = nc.dram_tensor("xhit", [KT, PER], F16, kind="ExternalInput").ap()
    xlot = nc.dram_tensor("xlot", [KT, PER], F8E3, kind="ExternalInput").ap()
    # fp16 consts: w1-hi chunks at cols [50c, 50c+50) (tail chunk at c=6,
    # rows 0:16), w1-lo (pre-scaled by 2^-12) at [350+50c, ...), w2 at
    # [700, 750), w3 at [750, 760); w2/w3 replicated at partition 64 for
    # the column-tiled pair.
    cb16 = nc.dram_tensor("cb16", [128, 760], F16, kind="ExternalInput").ap()
    # fp32 consts: col 0 = -T1, col 1 = -T2 (replicated at partition 64)
    cb32 = nc.dram_tensor("cb32", [128, 2], F32, kind="ExternalInput").ap()
    out = nc.dram_tensor("out", [PER, NF], F32, kind="ExternalOutput").ap()

    W2C = 700
    W3C = 750

    with TileContext(nc) as tc:
        with (
            tc.tile_pool(name="consts", bufs=1) as cpool,
            tc.tile_pool(name="xin", bufs=PF + 3) as xpool,
            tc.tile_pool(name="mid", bufs=3) as mpool,
            tc.tile_pool(name="fin", bufs=6) as fpool,
            tc.tile_pool(name="psA", bufs=2, space="PSUM") as psA,
            tc.tile_pool(name="psB", bufs=3, space="PSUM") as psB,
            tc.tile_pool(name="psW", bufs=1, space="PSUM") as psW,
        ):
            cb16t = cpool.tile([128, 760], F16, tag="cb16")
            nc.sync.dma_start(cb16t[:], cb16[:, :])
            cb32t = cpool.tile([128, 2], F32, tag="cb32")
            nc.scalar.dma_start(cb32t[:], cb32[:, :])
            # whole-core tail-feature tensors (loaded after the first two
            # slabs - their chunks run last in each accumulation)
            hit = cpool.tile([KT, PER], F16, tag="hit")
            lot = cpool.tile([KT, PER], F8E3, tag="lot")

            def emit_tail_consts():
                for c0 in range(0, PER, HICOL):
                    e = nc.sync if (c0 // HICOL) % 2 == 0 else nc.scalar
                    e.dma_start(hit[:, c0:c0 + HICOL], xhit[:, c0:c0 + HICOL])
                for c0 in range(0, PER, LOCOL):
                    e = nc.scalar if (c0 // LOCOL) % 2 == 0 else nc.sync
                    e.dma_start(lot[:, c0:c0 + LOCOL], xlot[:, c0:c0 + LOCOL])

            w1hi = [cb16t[0:128, c * NHID:(c + 1) * NHID] for c in range(NCH)]
            w1hit = cb16t[0:KT, NCH * NHID:(NCH + 1) * NHID]
            w1lo = [cb16t[0:128, 350 + c * NHID:350 + (c + 1) * NHID]
                    for c in range(NCH)]
            w1lot = cb16t[0:KT, 350 + NCH * NHID:350 + (NCH + 1) * NHID]
            w2t = cb16t[0:NHID, W2C:W2C + NHID]
            w3t = cb16t[0:NHID, W3C:W3C + NCLS]
            w2t64 = cb16t[64:64 + NHID, W2C:W2C + NHID]
            w3t64 = cb16t[64:64 + NHID, W3C:W3C + NCLS]
            nt1t = cb32t[0:64 + NHID, 0:1]
            nt2t = cb32t[0:64 + NHID, 1:2]

            # HAM keeps the PE array clock-gated to 1.2 GHz until it has
            # seen ~3.4us of sustained matmul activity; burn that in on
            # junk data during the DMA fill window so the real pipeline
            # runs at 2.4 GHz from slab 0.
            psw = psW.tile([64, 512], F32, tag="psw")
            for _ in range(NWARM):
                nc.tensor.matmul(psw[0:NHID, :], w2t, cb16t[0:NHID, 0:512],
                                 start=True, stop=True, skip_group_check=True)

            xt = {}
            s1t = {}
            s2v = {}
            ring = [0]

            def emit_loads(s):
                n = SLABS[s]
                off = NCH * B0[s]
                tot = NCH * n
                engs = (nc.sync, nc.scalar)
                th = xpool.tile([128, NCH * 1024], F16, tag="xh",
                                name=f"xh{s}")
                tl = xpool.tile([128, NCH * 1024], F8E3, tag="xl",
                                name=f"xl{s}")
                for c0 in range(0, tot, HICOL):
                    c1 = min(c0 + HICOL, tot)
                    engs[ring[0] % 2].dma_start(th[:, c0:c1],
                                                xhi[:, off + c0:off + c1])
                    ring[0] += 1
                for c0 in range(0, tot, LOCOL):
                    c1 = min(c0 + LOCOL, tot)
                    engs[ring[0] % 2].dma_start(tl[:, c0:c1],
                                                xlo[:, off + c0:off + c1])
                    ring[0] += 1
                xt[s] = (th, tl)

            def stageA(p):
                # one slab = 2 groups of n/2 rows run CONCURRENTLY on the PE
                # via column tiling: group 0 on array columns 0-63 (out
                # partitions 0-49), group 1 on columns 64-127 (out
                # partitions 64-113). 6+1 fp16-hi chunks then 6+1 fp8-lo
                # chunks accumulate into one PSUM group.
                n = SLABS[p]
                g = n // 2
                b0 = B0[p]
                th, tl = xt[p]
                ps1 = psA.tile([128, 512], F32, tag="ps1")
                hi_ops = [(w1hi[c], th[0:128, c * n:c * n + g],
                           th[0:128, c * n + g:(c + 1) * n])
                          for c in range(NCH)]
                lo_ops = [(w1lo[c], tl[0:128, c * n:c * n + g],
                           tl[0:128, c * n + g:(c + 1) * n])
                          for c in range(NCH)]
                ops = hi_ops + lo_ops + [
                    (w1hit, hit[:, b0:b0 + g], hit[:, b0 + g:b0 + n]),
                    (w1lot, lot[:, b0:b0 + g], lot[:, b0 + g:b0 + n])]
                for i, (w, ma, mb) in enumerate(ops):
                    nc.tensor.matmul(ps1[0:NHID, 0:g], w, ma,
                                     start=(i == 0), stop=(i == len(ops) - 1),
                                     skip_group_check=True)
                    nc.tensor.matmul(ps1[64:64 + NHID, 0:g], w, mb,
                                     start=(i == 0), stop=(i == len(ops) - 1),
                                     skip_group_check=True)
                s1 = mpool.tile([64 + NHID, 512], F16, tag="s1", name=f"s1_{p}")
                nc.scalar.sign(s1[:, 0:g], ps1[0:64 + NHID, 0:g], bias=nt1t)
                s1t[p] = (s1[0:NHID, 0:g], s1[64:64 + NHID, 0:g])

            def stageB(p):
                g = SLABS[p] // 2
                ps2 = psA.tile([128, 512], F32, tag="ps2")
                sa, sb = s1t[p]
                nc.tensor.matmul(ps2[0:NHID, 0:g], w2t, sa,
                                 start=True, stop=True, skip_group_check=True)
                nc.tensor.matmul(ps2[64:64 + NHID, 0:g], w2t64, sb,
                                 start=True, stop=True, skip_group_check=True)
                s2 = mpool.tile([64 + NHID, 512], F16, tag="s2", name=f"s2_{p}")
                nc.scalar.sign(s2[:, 0:g], ps2[0:64 + NHID, 0:g], bias=nt2t)
                v = s2[:, 0:g].rearrange("q (j r) -> q j r", r=8)
                s2v[p] = (v[0:NHID, :, :], v[64:64 + NHID, :, :])

            def stageCD(p):
                # Layer 3 fused with the output transpose: the stationary
                # operand is a stride-8 batch pick of s2, so out partition q
                # holds batch rows {8q + r} of the slab -> contiguous
                # batch-major store, no PSUM->SBUF logit copy and no PE
                # transpose pass.
                n = SLABS[p]
                g = n // 2
                j = g // 8
                ps4 = psB.tile([128, 8 * NCLS], F32, tag="ps4", name=f"ps4_{p}")
                s2a3, s2b3 = s2v[p]
                for r in range(8):
                    nc.tensor.matmul(ps4[0:j, r * NCLS:(r + 1) * NCLS],
                                     s2a3[:, :, r], w3t,
                                     start=True, stop=True,
                                     skip_group_check=True)
                    nc.tensor.matmul(ps4[64:64 + j, r * NCLS:(r + 1) * NCLS],
                                     s2b3[:, :, r], w3t64,
                                     start=True, stop=True,
                                     skip_group_check=True)
                eo = fpool.tile([128, 8 * NCLS], F32, tag="eo", name=f"eo_{p}")
                nc.scalar.activation(eo[:], ps4[:],
                                     mybir.ActivationFunctionType.Exp)
                sm = fpool.tile([128, 8], F32, tag="sm", name=f"sm_{p}")
                eov = eo[:].rearrange("q (r c) -> q r c", c=NCLS)
                nc.vector.tensor_reduce(sm[:], eov, axis=mybir.AxisListType.X,
                                        op=mybir.AluOpType.add)
                rv = fpool.tile([128, 8], F32, tag="rv", name=f"rv_{p}")
                nc.vector.reciprocal(rv[:], sm[:])
                ot = fpool.tile([128, 8 * NF], F32, tag="ot", name=f"ot_{p}")
                otv = ot[:].rearrange("q (r c) -> q r c", c=NF)
                rvb = rv[:].unsqueeze(-1).broadcast_to([128, 8, NCLS])
                nc.vector.tensor_mul(otv[:, :, 0:NCLS], eov, rvb)
                b0 = B0[p]
                eng = nc.sync if p % 2 == 0 else nc.scalar
                if n == 1024:
                    dst = out[b0:b0 + n, :].rearrange("(q r) f -> q (r f)",
                                                      q=128)
                    eng.dma_start(dst, ot[:])
                else:
                    # group 1 lives at partitions 0..j-1, group 2 at 64..63+j
                    d1 = out[b0:b0 + g, :].rearrange("(q r) f -> q (r f)", q=j)
                    d2 = out[b0 + g:b0 + n, :].rearrange("(q r) f -> q (r f)",
                                                         q=j)
                    eng.dma_start(d1, ot[0:j, :])
                    eng.dma_start(d2, ot[64:64 + j, :])

            # software pipeline: A(p) streams while ScalarE signs slab p-1
            # and the L3/softmax of slab p-2 fills PE/DVE slack; the two
            # 512-row tail slabs keep the post-stream drain short.
            emit_loads(0)
            emit_loads(1)
            emit_tail_consts()
            for s in range(2, PF):
                emit_loads(s)
            # loads are emitted LAST in each tick: the tick's ACTIVATEs
            # must precede them in the scalar FIFO, since a dma_start can
            # block on HWDGE flow control and convoy everything behind it
            for p in range(NSLAB - 2):
                stageA(p)
                if p >= 1:
                    stageB(p - 1)
                if p >= 2:
                    stageCD(p - 2)
                if p in (1, 2):
                    # the issue-release trickle after the upfront burst
                    # leaves the PE idle ~5us waiting for slab 2/3, which
                    # re-throttles the HAM clock right as the steady state
                    # begins; fill exactly that hole with junk matmuls so
                    # the pipeline enters the stream at 2.4 GHz
                    for _ in range(10 if p == 1 else 4):
                        nc.tensor.matmul(psw[0:NHID, :], w2t,
                                         cb16t[0:NHID, 0:512],
                                         start=True, stop=True,
                                         skip_group_check=True)
                if p + PF < NSLAB:
                    emit_loads(p + PF)
            stageA(NSLAB - 2)
            stageB(NSLAB - 3)
            stageCD(NSLAB - 4)
            stageA(NSLAB - 1)
            stageB(NSLAB - 2)
            stageCD(NSLAB - 3)
            stageB(NSLAB - 1)
            stageCD(NSLAB - 2)
            stageCD(NSLAB - 1)

    nc.compile()
    return nc


def _swizzle(xt, dtype):
    """[768, 8192] feature-major block (main chunks) -> [128, 6*8192]
    slab-blocked layout: per slab s a contiguous [128, 6*len] block whose
    column j = c*len + i holds feature c*128+p, batch B0[s]+i."""
    r = xt.reshape(NCH, 128, PER)
    blocks = [np.ascontiguousarray(
        r[:, :, B0[s]:B0[s] + SLABS[s]].transpose(1, 0, 2)
    ).reshape(128, NCH * SLABS[s]) for s in range(NSLAB)]
    return np.ascontiguousarray(np.concatenate(blocks, axis=1), dtype=dtype)


def _prep_host(inputs, W1, W2, W3, g1, b1, m1, v1, g2, b2, m2, v2):
    x = np.ascontiguousarray(inputs.reshape(B, K).astype(np.float32, copy=False))
    xhi = x.astype(np.float16)
    xlo8 = ((x - xhi.astype(np.float32)) * float(2 ** LOSHIFT)) \
        .astype(ml_dtypes.float8_e3m4)

    w1b = np.where(W1 >= 0, 1.0, -1.0).astype(np.float16)
    w2b = np.where(W2 >= 0, 1.0, -1.0).astype(np.float16)
    w3b = np.where(W3 >= 0, 1.0, -1.0).astype(np.float16)

    a1 = g1.astype(np.float64) / np.sqrt(v1.astype(np.float64) + EPS)
    c1 = b1.astype(np.float64) - a1 * m1.astype(np.float64)
    t1 = -c1 / a1
    T1 = np.where(t1 > 0, t1, -1e30).astype(np.float32)
    a2 = g2.astype(np.float64) / np.sqrt(v2.astype(np.float64) + EPS)
    c2 = b2.astype(np.float64) - a2 * m2.astype(np.float64)
    t2 = -c2 / a2
    T2 = np.where(t2 > 0, t2, -1e30).astype(np.float32)

    lscale = np.float16(2.0 ** -LOSHIFT)
    cb16 = np.zeros((128, 760), dtype=np.float16)
    for c in range(NCH):
        cb16[:, c * NHID:(c + 1) * NHID] = w1b[c * 128:(c + 1) * 128]
        cb16[:, 350 + c * NHID:350 + (c + 1) * NHID] = \
            w1b[c * 128:(c + 1) * 128] * lscale
    cb16[:KT, NCH * NHID:(NCH + 1) * NHID] = w1b[NCH * 128:]
    cb16[:KT, 350 + NCH * NHID:350 + (NCH + 1) * NHID] = w1b[NCH * 128:] * lscale
    cb16[:NHID, 700:750] = w2b
    cb16[:NHID, 750:760] = w3b
    cb16[64:64 + NHID, 700:750] = w2b
    cb16[64:64 + NHID, 750:760] = w3b
    cb32 = np.zeros((128, 2), dtype=np.float32)
    cb32[:NHID, 0] = -T1
    cb32[64:64 + NHID, 0] = -T1
    cb32[:NHID, 1] = -T2
    cb32[64:64 + NHID, 1] = -T2
    shared = {"cb16": cb16, "cb32": cb32}
    in_maps = []
    for c in range(NCORES):
        sl = slice(c * PER, (c + 1) * PER)
        m = dict(shared)
        hiT = np.ascontiguousarray(xhi[sl].T)       # [784, 8192]
        loT = np.ascontiguousarray(xlo8[sl].T)
        m["xhi"] = _swizzle(hiT[:NCH * 128], np.float16)
        m["xlo"] = _swizzle(loT[:NCH * 128], ml_dtypes.float8_e3m4)
        m["xhit"] = np.ascontiguousarray(hiT[NCH * 128:])
        m["xlot"] = np.ascontiguousarray(loT[NCH * 128:])
        in_maps.append(m)
    return in_maps


def kernel(**inputs):
    if "nc" not in _CACHE:
        _CACHE["nc"] = _build()
    nc = _CACHE["nc"]
    inputs = {k: np.asarray(v) for k, v in inputs.items()}
    in_maps = _prep_host(**inputs)
    res = run_bass_kernel_spmd(nc, in_maps, core_ids=list(range(NCORES)))
    return np.ascontiguousarray(
        np.concatenate([r["out"] for r in res.results], axis=0)[:, :NCLS])
